# revision 1
# baseline (speedup 1.0000x reference)
"""Multi-head attention (B=2, T=2048, E=2048, H=16) on 8 trn2 NeuronCores.

Sharding: core c handles batch b = c//4 and head-group g = c%4 (4 heads,
512 of the 2048 projection dims). Each core computes its heads' QKV
projections, attention, and a partial out-projection over its 512 context
dims; the host sums the 4 partials per batch and adds the output bias.

Per-core pipeline (all matmuls fp32r = full-rate fp32 on the PE):
  1. Q^T/K^T = Wg @ x^T (plus bias), staged to DRAM scratch.
     V = x @ Wv_g^T (plus bias) in [token, dim] layout, staged to DRAM.
  2. Per (head, 512-query block): S^T[k,q] = K_h @ Q_h^T; P^T = exp(S^T*scale)
     unnormalized; row sums via ones-matmul; ctx^T = V_h^T @ P^T; normalize
     by broadcasting 1/sum across partitions (GPSIMD partition_broadcast).
     Softmax skips the max-subtraction: scores here are O(10), far from
     fp32 exp overflow, so the result is mathematically identical.
  3. out_partial = ctx^T.T @ Wo_g^T, DMA'd straight to the output.
"""

import numpy as np

from concourse import bacc
import concourse.mybir as mybir
import concourse.tile as tile
from concourse.bass_utils import run_bass_kernel_spmd

B, T, E = 2, 2048, 2048
H, D = 16, 128
NCORES, GROUPS = 8, 4
HL = H // GROUPS            # heads per core
M = HL * D                  # 512 local projection dims
P = 128
KT = E // P                 # 16 contraction tiles over E
MT = M // P                 # 4
NT = T // 512               # 4 t-slices of 512
F32 = mybir.dt.float32
F32R = mybir.dt.float32r
EXP = mybir.ActivationFunctionType.Exp
SCALE = float(1.0 / np.sqrt(D))


def build_nc(reps=1, phases="123"):
    nc = bacc.Bacc()
    xT = nc.declare_dram_parameter("xT", [E, T], F32, isOutput=False)
    wq = nc.declare_dram_parameter("wq", [E, M], F32, isOutput=False)
    wk = nc.declare_dram_parameter("wk", [E, M], F32, isOutput=False)
    wv = nc.declare_dram_parameter("wv", [E, M], F32, isOutput=False)
    wo = nc.declare_dram_parameter("wo", [M, E], F32, isOutput=False)
    bqT = nc.declare_dram_parameter("bqT", [P, MT], F32, isOutput=False)
    bkT = nc.declare_dram_parameter("bkT", [P, MT], F32, isOutput=False)
    bvb = nc.declare_dram_parameter("bvb", [P, M], F32, isOutput=False)
    kbias = nc.declare_dram_parameter("kbias", [P, KT], F32, isOutput=False)
    onesd = nc.declare_dram_parameter("onesd", [P, 1], F32, isOutput=False)
    out = nc.declare_dram_parameter("out", [T, E], F32, isOutput=True)

    # DRAM scratch for phase staging
    qTd = nc.dram_tensor("qTd", [M, T], F32)
    kTd = nc.dram_tensor("kTd", [M, T], F32)
    vd = nc.dram_tensor("vd", [T, M], F32)
    ctxTd = nc.dram_tensor("ctxTd", [M, T], F32)

    xT_r = xT.bitcast(F32R).rearrange("(k p) t -> p k t", p=P)
    wq_r = wq.bitcast(F32R).rearrange("(k p) m -> p k m", p=P)
    wk_r = wk.bitcast(F32R).rearrange("(k p) m -> p k m", p=P)
    wv_r = wv.bitcast(F32R).rearrange("(k p) m -> p k m", p=P)
    wo_r = wo.bitcast(F32R).rearrange("(c p) e -> p c e", p=P)
    qTd_w = qTd.rearrange("(m p) t -> p m t", p=P)
    kTd_w = kTd.rearrange("(m p) t -> p m t", p=P)
    qTd_r = qTd.bitcast(F32R).rearrange("(m p) t -> p m t", p=P)
    kTd_r = kTd.bitcast(F32R).rearrange("(m p) t -> p m t", p=P)
    vd_w = vd.rearrange("(tt p) m -> p tt m", p=P)
    vd_r = vd.bitcast(F32R).rearrange("(tt p) m -> p tt m", p=P)
    ctxTd_w = ctxTd.rearrange("(m p) t -> p m t", p=P)
    ctxTd_r = ctxTd.bitcast(F32R).rearrange("(m p) t -> p m t", p=P)
    out_w = out.rearrange("(tt p) e -> p tt e", p=P)

    ts = lambda i, s: slice(i * s, (i + 1) * s)

    with tile.TileContext(nc) as tc:
        with (
            tc.tile_pool(name="const", bufs=1) as cpool,
            tc.tile_pool(name="psum", bufs=1, space="PSUM") as psum,
        ):
            bq_s = cpool.tile([P, MT], F32, tag="bq")
            bk_s = cpool.tile([P, MT], F32, tag="bk")
            bv_s = cpool.tile([P, M], F32, tag="bv")
            kb_s = cpool.tile([P, KT], F32, tag="kb")
            ones = cpool.tile([P, 1], F32R, tag="ones")
            nc.sync.dma_start(bq_s[:], bqT[:])
            nc.sync.dma_start(bk_s[:], bkT[:])
            nc.sync.dma_start(bv_s[:], bvb[:])
            nc.sync.dma_start(kb_s[:], kbias[:])
            nc.sync.dma_start(ones[:], onesd[:].bitcast(F32R))

            for _ in range(reps):
                with (
                    tc.tile_pool(name="w", bufs=1) as wpool,
                    tc.tile_pool(name="xn", bufs=2) as xpool,
                    tc.tile_pool(name="stage", bufs=2) as stpool,
                ):
                    # ---- phase 1: Q^T/K^T = W @ x^T, V = x @ Wv^T (+biases) ----
                    wq_s = wpool.tile([P, KT, M], F32R, tag="wq")
                    wk_s = wpool.tile([P, KT, M], F32R, tag="wk")
                    wv_s = wpool.tile([P, KT, M], F32R, tag="wv")
                    for k in range(0, KT, 4):
                        nc.sync.dma_start(wq_s[:, k:k + 4], wq_r[:, k:k + 4])
                        nc.sync.dma_start(wk_s[:, k:k + 4], wk_r[:, k:k + 4])
                        nc.sync.dma_start(wv_s[:, k:k + 4], wv_r[:, k:k + 4])
                    for n in range(NT):
                        xn = xpool.tile([P, KT, 512], F32R, tag="xn")
                        for k in range(0, KT, 4):
                            nc.sync.dma_start(xn[:, k:k + 4], xT_r[:, k:k + 4, ts(n, 512)])
                        for w_s, b_s, dst in ((wq_s, bq_s, qTd_w), (wk_s, bk_s, kTd_w)):
                            st = stpool.tile([P, MT, 512], F32, tag="stqk")
                            for m in range(MT):
                                ps = psum.tile([P, 512], F32, tag="mm", bufs=4)
                                for k in range(KT):
                                    nc.tensor.matmul(ps[:], w_s[:, k, ts(m, P)],
                                                     xn[:, k],
                                                     start=(k == 0), stop=(k == KT - 1))
                                nc.vector.tensor_scalar_add(st[:, m], ps[:], b_s[:, m:m + 1])
                            nc.scalar.dma_start(dst[:, :, ts(n, 512)], st[:])
                        stv = stpool.tile([P, 4, 512], F32, tag="stv")
                        for t in range(4):
                            ps = psum.tile([P, 512], F32, tag="mm", bufs=4)
                            for k in range(KT):
                                nc.tensor.matmul(ps[:], xn[:, k, ts(t, P)], wv_s[:, k],
                                                 start=(k == 0), stop=(k == KT - 1))
                            nc.vector.tensor_add(out=stv[:, t], in0=ps[:], in1=bv_s[:])
                        nc.scalar.dma_start(vd_w[:, ts(n, 4)], stv[:])

                # ---- phase 2: attention per (head, 512-query block) ----
                with (
                    tc.tile_pool(name="wo", bufs=1) as wopool,
                    tc.tile_pool(name="ctx", bufs=1) as cxpool,
                    tc.tile_pool(name="attn", bufs=2) as apool,
                    tc.tile_pool(name="es", bufs=2) as espool,
                    tc.tile_pool(name="small", bufs=2) as smpool,
                ):
                    wo_s = wopool.tile([P, MT, E], F32R, tag="wo")
                    nc.sync.dma_start(wo_s[:], wo_r[:])
                    ctx_s = cxpool.tile([P, MT, T], F32R, tag="ctx")
                    for h in range(HL):
                        kTh = apool.tile([P, T], F32R, tag="kTh")
                        qTh = apool.tile([P, T], F32R, tag="qTh")
                        vh = apool.tile([P, KT, P], F32R, tag="vh")
                        nc.sync.dma_start(kTh[:], kTd_r[:, h])
                        nc.sync.dma_start(qTh[:], qTd_r[:, h])
                        nc.sync.dma_start(vh[:], vd_r[:, :, ts(h, P)])
                        for qb in range(NT):
                            es = espool.tile([P, KT, 512], F32R, tag="es")
                            for kt in range(KT):
                                ps = psum.tile([P, 512], F32, tag="mm", bufs=4)
                                nc.tensor.matmul(ps[:], kTh[:, ts(kt, P)],
                                                 qTh[:, ts(qb, 512)],
                                                 start=True, stop=True)
                                nc.scalar.activation(es[:, kt], ps[:], EXP,
                                                     bias=kb_s[:, kt:kt + 1],
                                                     scale=SCALE)
                            sps = psum.tile([1, 512], F32, tag="sum", bufs=2)
                            for kt in range(KT):
                                nc.tensor.matmul(sps[:], ones[:], es[:, kt],
                                                 start=(kt == 0), stop=(kt == KT - 1))
                            aps = psum.tile([P, 512], F32, tag="av", bufs=2)
                            for kt in range(KT):
                                nc.tensor.matmul(aps[:], vh[:, kt], es[:, kt],
                                                 start=(kt == 0), stop=(kt == KT - 1))
                            row = smpool.tile([1, 512], F32, tag="row")
                            nc.vector.tensor_copy(row[:], sps[:])
                            bc = smpool.tile([P, 512], F32, tag="bc")
                            nc.gpsimd.partition_broadcast(bc[:], row[:])
                            rc = smpool.tile([P, 512], F32, tag="rc")
                            nc.vector.reciprocal(rc[:], bc[:])
                            nc.vector.tensor_mul(out=ctx_s[:, h, ts(qb, 512)],
                                                 in0=aps[:], in1=rc[:])

                    # ---- phase 3: out_partial = ctx^T.T @ Wo^T ----
                    for tt in range(KT if "3" in phases else 0):
                        st = smpool.tile([P, NT, 512], F32, tag="stout")
                        for e in range(NT):
                            ps = psum.tile([P, 512], F32, tag="mm", bufs=4)
                            for c in range(MT):
                                nc.tensor.matmul(ps[:], ctx_s[:, c, ts(tt, P)],
                                                 wo_s[:, c, ts(e, 512)],
                                                 start=(c == 0), stop=(c == MT - 1))
                            nc.any.tensor_copy(out=st[:, e], in_=ps[:])
                        nc.scalar.dma_start(out_w[:, tt], st[:])

    nc.compile()
    return nc


_cache = {}


def _get_nc(reps=1):
    if reps not in _cache:
        _cache[reps] = build_nc(reps)
    return _cache[reps]


def make_in_maps(x, mask, Wq, bq, Wk, bk, Wv, bv, Wo, bo):
    in_maps = []
    for c in range(NCORES):
        b, g = divmod(c, GROUPS)
        sl = slice(g * M, (g + 1) * M)
        kb = np.where(np.asarray(mask[b]), 0.0, -10000.0).astype(np.float32)
        in_maps.append({
            "xT": np.ascontiguousarray(np.asarray(x[b]).T),
            "wq": np.ascontiguousarray(np.asarray(Wq[sl]).T),
            "wk": np.ascontiguousarray(np.asarray(Wk[sl]).T),
            "wv": np.ascontiguousarray(np.asarray(Wv[sl]).T),
            "wo": np.ascontiguousarray(np.asarray(Wo[:, sl]).T),
            "bqT": np.ascontiguousarray(np.asarray(bq[sl]).reshape(MT, P).T),
            "bkT": np.ascontiguousarray(np.asarray(bk[sl]).reshape(MT, P).T),
            "bvb": np.ascontiguousarray(
                np.broadcast_to(np.asarray(bv[sl]), (P, M))),
            "kbias": np.ascontiguousarray(kb.reshape(KT, P).T),
            "onesd": np.ones((P, 1), dtype=np.float32),
        })
    return in_maps


def combine(results, bo):
    out = np.empty((B, T, E), dtype=np.float32)
    for b in range(B):
        acc = results[b * GROUPS]["out"].astype(np.float32).copy()
        for g in range(1, GROUPS):
            acc += results[b * GROUPS + g]["out"]
        out[b] = acc + np.asarray(bo)
    return out


def kernel(x, mask, Wq, bq, Wk, bk, Wv, bv, Wo, bo):
    nc = _get_nc(1)
    in_maps = make_in_maps(x, mask, Wq, bq, Wk, bk, Wv, bv, Wo, bo)
    res = run_bass_kernel_spmd(nc, in_maps, list(range(NCORES)))
    return combine(res.results, bo)

